# revision 43
# baseline (speedup 1.0000x reference)
import numpy as np

# nn_MultiHeadedAttention: B=4, S=2048, D_MODEL=1024, H=16, D_K=64, fp32.
# Sharding: 8 cores = 4 batches x 2 head-groups (8 heads each).
# All-bf16 pipeline (fp8 DoubleRow fails the 2e-2 gate at peaked-softmax
# rows; bf16 keeps rel err ~9e-3): bf16 projections/scores/PV/out-proj,
# exp on ACT straight out of PSUM into bf16, softmax denominator via an
# appended ones column in V, fused PE-broadcast of both head reciprocals,
# bf16 partial y summed on host.

B, S, D, H, DK = 4, 2048, 1024, 16, 64
NCORES = 8
DG = 512  # dims per head-group (8 heads x 64)

_NC_CACHE = {}
LAST_EXEC_NS = None


def _build_nc():
    import concourse.bacc as bacc
    import concourse.tile as tile
    from concourse import mybir

    F32 = mybir.dt.float32
    BF16 = mybir.dt.bfloat16
    EXP = mybir.ActivationFunctionType.Exp

    nc = bacc.Bacc(None, target_bir_lowering=False, debug=True)

    xqT = nc.dram_tensor("xqT", [D, S], BF16, kind="ExternalInput")
    xkT = nc.dram_tensor("xkT", [D, S], BF16, kind="ExternalInput")
    xvT = nc.dram_tensor("xvT", [D, S], BF16, kind="ExternalInput")
    wqT = nc.dram_tensor("wqT", [D, DG], BF16, kind="ExternalInput")
    wkT = nc.dram_tensor("wkT", [D, DG], BF16, kind="ExternalInput")
    wvT = nc.dram_tensor("wvT", [D, DG], BF16, kind="ExternalInput")
    woT = nc.dram_tensor("woT", [DG, D], BF16, kind="ExternalInput")
    bqc = nc.dram_tensor("bqc", [128, 4], F32, kind="ExternalInput")
    bkc = nc.dram_tensor("bkc", [128, 4], F32, kind="ExternalInput")
    bvr = nc.dram_tensor("bvr", [DG], F32, kind="ExternalInput")
    y_d = nc.dram_tensor("y", [S, D], BF16, kind="ExternalOutput")

    import concourse.bass as bass

    with (
        tile.TileContext(nc) as tc,
        nc.allow_low_precision(reason="bf16 within rel-err budget"),
        tc.tile_pool(name="persist", bufs=1) as persist,
        tc.tile_pool(name="stage", bufs=2) as stage,
        tc.tile_pool(name="asb", bufs=3) as asb,
        tc.tile_pool(name="ps_st", bufs=2, space="PSUM") as ps_st,
        tc.tile_pool(name="ps_ab", bufs=2, space="PSUM") as ps_ab,
    ):
        _abn = [0]

        def ab_tile():
            _abn[0] += 1
            return ps_ab.tile([128, 512], F32, name=f"ab{_abn[0] % 2}", bufs=2)
        QT = [persist.tile([128, S], BF16, name=f"QT{p}") for p in range(4)]
        KT = [persist.tile([128, S], BF16, name=f"KT{p}") for p in range(4)]
        AT = [persist.tile([128, S], BF16, name=f"AT{p}") for p in range(4)]
        VO = [persist.tile([128, 8, 65], BF16, name=f"VO{s}") for s in range(16)]
        bq_sb = persist.tile([128, 4], F32, name="bq_sb")
        bk_sb = persist.tile([128, 4], F32, name="bk_sb")
        bv_sb = persist.tile([128, DG], F32, name="bv_sb")
        ones1 = persist.tile([1, 128], BF16, name="ones1")

        nc.gpsimd.dma_start(bq_sb[:], bqc[:])
        nc.gpsimd.dma_start(bk_sb[:], bkc[:])
        bv_ap = bvr[:]
        bv_bcast = bass.AP(tensor=bv_ap.tensor, offset=bv_ap.offset, ap=[[0, 128], *bv_ap.ap])
        nc.gpsimd.dma_start(bv_sb[:], bv_bcast)
        nc.vector.memset(ones1[:], 1.0)
        for s in range(16):
            nc.vector.memset(VO[s][:, :, 64:65], 1.0)

        # ---- Q, K, V projections ----
        if True:
            def load_w(w_d):
                wt = stage.tile([128, 8, DG], BF16, name="wt")
                for i in range(8):
                    nc.gpsimd.dma_start(wt[:, i, :], w_d[i * 128 : (i + 1) * 128, :])
                return wt

            def qk_phase(x_d, b_sb, wt, OUT):
                for qc in range(4):
                    qs = slice(qc * 512, (qc + 1) * 512)
                    xs = stage.tile([128, 8, 512], BF16, name="xs")
                    for i in range(8):
                        eng = nc.sync if i % 2 == 0 else nc.scalar
                        eng.dma_start(xs[:, i, :], x_d[i * 128 : (i + 1) * 128, qs])
                    for p in range(4):
                        pp = ab_tile()
                        for i in range(8):
                            nc.tensor.matmul(
                                pp[:],
                                wt[:, i, p * 128 : (p + 1) * 128],
                                xs[:, i, :],
                                start=(i == 0),
                                stop=(i == 7),
                            )
                        nc.vector.tensor_scalar_add(OUT[p][:, qs], pp[:], b_sb[:, p : p + 1])

            wq = load_w(wqT)
            wk = load_w(wkT)
            qk_phase(xqT, bq_sb, wq, QT)
            wv = load_w(wvT)  # reuses wq's buffer; WAR on Q reads already recorded
            qk_phase(xkT, bk_sb, wk, KT)

        def v_phase():
            for sb in range(16):
                ss = slice(sb * 128, (sb + 1) * 128)
                xv = stage.tile([128, 8, 128], BF16, name="xv", bufs=3)
                for i in range(8):
                    eng = nc.sync if i % 2 == 0 else nc.scalar
                    eng.dma_start(xv[:, i, :], xvT[i * 128 : (i + 1) * 128, ss])
                vp = ab_tile()
                for i in range(8):
                    nc.tensor.matmul(
                        vp[:], xv[:, i, :], wv[:, i, :], start=(i == 0), stop=(i == 7)
                    )
                nc.vector.tensor_add(
                    VO[sb][:, :, 0:64],
                    vp[:].rearrange("p (h d) -> p h d", h=8),
                    bv_sb[:].rearrange("p (h d) -> p h d", h=8),
                )

        # prefetch out-proj weights during attention (gpsimd idle there)
        owp_cm = tc.tile_pool(name="out_w", bufs=1)
        owp = owp_cm.__enter__()
        wo = owp.tile([128, 4, D], BF16, name="wo")
        for p in range(4):
            nc.gpsimd.dma_start(wo[:, p, :], woT[p * 128 : (p + 1) * 128, :])

        # emit unit (0,0)'s score chunks now so ACT exp overlaps the V phase
        pre_ech = {(0, 0): []}
        for c in range(4):
            ech = asb.tile([128, 2, 4, 512], BF16, name="ech", bufs=4)
            pre_ech[(0, 0)].append(ech)
            for kbi in range(4):
                kb = c * 4 + kbi
                st = ps_st.tile([128, 2, 512], F32, name="st")
                for h in range(2):
                    nc.tensor.matmul(
                        st[:, h, :],
                        KT[0][h * 64 : (h + 1) * 64, kb * 128 : (kb + 1) * 128],
                        QT[0][h * 64 : (h + 1) * 64, 0:512],
                        start=True,
                        stop=True,
                    )
                nc.scalar.activation(
                    out=ech[:, :, kbi, :], in_=st[:], func=EXP, scale=0.125
                )
        v_phase()

        # ---- attention: scores -> exp(bf16) -> PV -> normalize ----
        if True:
            def emit_norm(state):
                pp, pqs, ppv, prec2 = state
                bc = ps_st.tile([128, 2, 512], F32, name="st")
                for h in range(2):
                    nc.tensor.matmul(
                        bc[:, h, :], ones1[:], prec2[:, h, :], start=True, stop=True
                    )
                bcs = asb.tile([128, 512], BF16, name="bcs", bufs=2)
                for h in range(2):
                    hb = h * 64
                    nc.vector.tensor_copy(bcs[hb : hb + 64, :], bc[hb : hb + 64, h, :])
                for h in range(2):
                    hb = h * 64
                    nc.vector.tensor_mul(
                        AT[pp][hb : hb + 64, pqs], ppv[h][0:64, :], bcs[hb : hb + 64, :]
                    )

            prev = None
            for p in range(4):
                for qc in range(4):
                    qs = slice(qc * 512, (qc + 1) * 512)
                    pv = [ab_tile() for h in range(2)]
                    ech_tiles = list(pre_ech.pop((p, qc), []))

                    def sc_chunk(c):
                        ech = asb.tile([128, 2, 4, 512], BF16, name="ech", bufs=4)
                        ech_tiles.append(ech)
                        for kbi in range(4):
                            kb = c * 4 + kbi
                            st = ps_st.tile([128, 2, 512], F32, name="st")
                            for h in range(2):
                                nc.tensor.matmul(
                                    st[:, h, :],
                                    KT[p][h * 64 : (h + 1) * 64, kb * 128 : (kb + 1) * 128],
                                    QT[p][h * 64 : (h + 1) * 64, qs],
                                    start=True,
                                    stop=True,
                                )
                            nc.scalar.activation(
                                out=ech[:, :, kbi, :], in_=st[:], func=EXP, scale=0.125
                            )

                    def pv_half(half):
                        for h in range(2):
                            hidx = p * 2 + h
                            for kbj in range(8):
                                c, kbi = divmod(kbj, 4)
                                kb = half * 8 + kbj
                                nc.tensor.matmul(
                                    pv[h][0:65, :],
                                    VO[kb][:, hidx, :],
                                    ech_tiles[half * 2 + c][:, h, kbi, :],
                                    start=(half == 0 and kbj == 0),
                                    stop=(half == 1 and kbj == 7),
                                )

                    if not ech_tiles:
                        sc_chunk(0)
                        sc_chunk(1)
                        sc_chunk(2)
                    pv_half(0)
                    if prev is not None:
                        emit_norm(prev)
                        prev = None
                    if len(ech_tiles) < 4:
                        sc_chunk(3)
                    pv_half(1)
                    rec2 = asb.tile([1, 2, 512], BF16, name="rec2", bufs=2)
                    for h in range(2):
                        nc.vector.reciprocal(rec2[:, h, :], pv[h][64:65, :])
                    prev = (p, qs, pv, rec2)
            emit_norm(prev)

        # ---- output projection (partial y for this head-group) ----
        if True:
            for sb in range(16):
                ss = slice(sb * 128, (sb + 1) * 128)
                ys = asb.tile([128, 2, 512], BF16, name="ys")
                for oc in range(2):
                    yp = ab_tile()
                    for p in range(4):
                        nc.tensor.matmul(
                            yp[:],
                            AT[p][:, ss],
                            wo[:, p, oc * 512 : (oc + 1) * 512],
                            start=(p == 0),
                            stop=(p == 3),
                        )
                    nc.vector.tensor_copy(ys[:, oc, :], yp[:])
                eng = (nc.gpsimd, nc.sync, nc.scalar)[sb % 3]
                eng.dma_start(y_d[ss, :], ys[:])
        owp_cm.__exit__(None, None, None)

    nc.compile()
    return nc


def _get_nc():
    if "nc" not in _NC_CACHE:
        _NC_CACHE["nc"] = _build_nc()
    return _NC_CACHE["nc"]


def kernel(**inputs):
    import ml_dtypes
    from concourse import bass_utils

    BF = ml_dtypes.bfloat16
    q, k, v = inputs["query"], inputs["key"], inputs["value"]
    Wq, Wk, Wv, Wo = inputs["Wq"], inputs["Wk"], inputs["Wv"], inputs["Wo"]
    bq, bk, bv, bo = inputs["bq"], inputs["bk"], inputs["bv"], inputs["bo"]

    nc = _get_nc()
    in_maps = []
    for c in range(NCORES):
        b, hg = divmod(c, 2)
        r0 = hg * DG
        rs = slice(r0, r0 + DG)
        in_maps.append(
            {
                "xqT": np.ascontiguousarray(q[b].T).astype(BF),
                "xkT": np.ascontiguousarray(k[b].T).astype(BF),
                "xvT": np.ascontiguousarray(v[b].T).astype(BF),
                "wqT": np.ascontiguousarray(Wq[rs, :].T).astype(BF),
                "wkT": np.ascontiguousarray(Wk[rs, :].T).astype(BF),
                "wvT": np.ascontiguousarray(Wv[rs, :].T).astype(BF),
                "woT": np.ascontiguousarray(Wo[:, rs].T).astype(BF),
                "bqc": np.ascontiguousarray(bq[rs].reshape(4, 128).T),
                "bkc": np.ascontiguousarray(bk[rs].reshape(4, 128).T),
                "bvr": np.ascontiguousarray(bv[rs]),
            }
        )
    import os

    trace = bool(os.environ.get("KERNEL_TRACE"))
    res = bass_utils.run_bass_kernel_spmd(
        nc, in_maps, core_ids=list(range(NCORES)), trace=trace
    )
    global LAST_EXEC_NS
    LAST_EXEC_NS = res.exec_time_ns
    out = np.empty((B, S, D), np.float32)
    for b in range(B):
        out[b] = (
            res.results[2 * b]["y"].astype(np.float32)
            + res.results[2 * b + 1]["y"].astype(np.float32)
            + bo[None, :]
        )
    return out
